# revision 56
# baseline (speedup 1.0000x reference)
"""Trainium2 Bass kernel for nn_Attention (B=4, S=1024, D=1024, H=16).

Sharding: 8 cores = 4 batches x 2 head-groups (tensor parallel over heads).
Core (b, g) computes the Q/K/V projections for its 512 features (8 heads),
full-sequence attention for those heads, and a PARTIAL output projection
(contraction over its 512 ctx features). Partials are written fp16 and the
two halves of each batch are summed on the host during gather (+ bias).
No device collectives.

Device dataflow (per core) - fp16 matmul operands, fp32 PSUM accumulation:
  - host passes pre-transposed qT/kT/vT [D,S] (partition-major), m-blocked
    W{q,k}.T halves [MT,P,NT,P], Wv.T half [P,NT,512], Wo.T half [P,MT,D]
  - qhT[o,sq] = (Wq.T*SCALE).T-tiles @ qT   (o on partitions, 4 m-tiles)
  - khT[o,sk] likewise; vh[sk, h, dh] via vT-as-stationary
  - scoresT[sk,sq] per head = khT-tile.T @ qhT; the two heads of a pair run
    as K=64 matmuls at PE row strips 0:64 / 64:128 (concurrent on HW via
    per-row-tile output taps: the pair costs ~321ns, second slice ~3ns)
  - expT = exp(scoresT) on ACT (no max subtraction: |scores| < ~4)
  - ctxT_aug[dh+1, sq] += [vh | 1].T @ expT  (ones column = denominator)
  - ctx drained fast on DVE (sum-row + approx-reciprocal first), gpsimd
    broadcast + normalize multiply later, off the critical path
  - partial out[sq,o] = ctxT-tiles.T @ Wo.T-half, drained to fp16, DMA'd

The attention phase is a flat software-pipelined (pair, q-half, key-tile)
loop with scores emitted two steps ahead. Projection-matmul filler work is
split into ~3-matmul units drained one per step (need-driven ordering), so
the PE runs dense through all 64 steps instead of front-loading whole
groups and going ACT-bound at the tail. All blocks except the last
normalize in-loop; the final block's reciprocals broadcast via a PE
ones-row matmul while the first 8 output-projection groups (which only
need q-cols 0:512) run, then the rest of the output projection follows.

Bias handling (exact): bq via per-partition add on the qh copy; bk dropped
(softmax is invariant to per-query score shifts); bv folded into bo on the
host (softmax rows sum to 1); bo added host-side after summing partials.
"""

import sys

import numpy as np

if "/opt/trn_rl_repo" not in sys.path:
    sys.path.insert(0, "/opt/trn_rl_repo")

B, S, D, H = 4, 1024, 1024, 16
HD = D // H                      # 64
SCALE = 1.0 / float(np.sqrt(HD))
N_CORES = 8
SQ = S                           # full query length per core
SK = S                           # full key length
P = 128
NT = D // P                      # 8 contraction tiles
MT = 4                           # 4 out-feature tiles (512 features/core)
SKT = SK // P                    # 8 key tiles
NPAIR = 4                        # 4 head pairs per core (8 heads)
NC2 = 512                        # max matmul free dim (one PSUM bank)
NCQ = SQ // NC2                  # 2 query chunks

_CACHE = {}


def _build_program():
    from contextlib import ExitStack

    import concourse.bass as bass
    import concourse.tile as tile
    from concourse import bacc, mybir

    F32 = mybir.dt.float32
    F16 = mybir.dt.float16
    AF = mybir.ActivationFunctionType

    nc = bacc.Bacc(
        "TRN2", target_bir_lowering=False, debug=False, num_devices=N_CORES
    )

    qT_d = nc.dram_tensor("qT", [NCQ, P, NT, NC2], F16,
                          kind="ExternalInput").ap()
    kT_d = nc.dram_tensor("kT", [NCQ, P, NT, NC2], F16,
                          kind="ExternalInput").ap()
    vT_d = nc.dram_tensor("vT", [P, SKT, NT, P], F16,
                          kind="ExternalInput").ap()
    wqT_d = nc.dram_tensor("wqT", [P, MT, NT, P], F16,
                           kind="ExternalInput").ap()
    wkT_d = nc.dram_tensor("wkT", [P, MT, NT, P], F16,
                           kind="ExternalInput").ap()
    wvT_d = nc.dram_tensor("wvT", [2, P, NT, NC2 // 2], F16,
                           kind="ExternalInput").ap()
    woT_d = nc.dram_tensor("woT", [P, MT, D], F16, kind="ExternalInput").ap()
    bq_d = nc.dram_tensor("bq", [MT * P], F32, kind="ExternalInput").ap()
    out_d = nc.dram_tensor("out", [SQ, D], F16, kind="ExternalOutput").ap()

    mm = lambda *a, **k: nc.tensor.matmul(*a, **k)

    with tile.TileContext(nc) as tc, ExitStack() as ctx:
        persist = ctx.enter_context(tc.tile_pool(name="persist", bufs=1))
        epool = ctx.enter_context(tc.tile_pool(name="epool", bufs=6))
        rpool = ctx.enter_context(tc.tile_pool(name="rp", bufs=2))
        opool = ctx.enter_context(tc.tile_pool(name="outp", bufs=4))
        pp = ctx.enter_context(tc.tile_pool(name="pp", space="PSUM", bufs=2))
        pS = ctx.enter_context(tc.tile_pool(name="pS", space="PSUM", bufs=2))
        pX = ctx.enter_context(tc.tile_pool(name="pX", space="PSUM", bufs=1))

        # persistent data tiles
        qT_sb = persist.tile([P, NCQ, NT, NC2], F16)  # q-chunk-major
        kT_sb = persist.tile([P, NCQ, NT, NC2], F16)
        vT_sb = persist.tile([P, SKT, NT, P], F16)   # key-tile-major
        wq = persist.tile([P, MT, NT, P], F16)       # m-major
        wk = persist.tile([P, MT, NT, P], F16)
        wv = persist.tile([P, 2, NT, NC2 // 2], F16)  # head-half-major
        wo = persist.tile([P, MT, D], F16)
        qhT = persist.tile([P, MT, SQ], F16)        # [o'%128, o'//128, sq]
        khT = persist.tile([P, MT, SK], F16)
        vh = persist.tile([P, SKT, 8, HD + 1], F16)  # [sk%128, sk//128, h, .]
        ctxT = persist.tile([P, MT, SQ], F16)
        ctxU = persist.tile([P, MT, SQ], F16)    # unnormalized ctx (drain)
        bq_sb = persist.tile([P, MT], F32)

        # input DMAs: every transfer is contiguous on BOTH DRAM and SBUF
        # sides (2-8KB runs per partition -> few descriptors), spread across
        # the four engines' DMA paths in per-engine deadline order.
        # No pre-loop DMAs on the scalar engine: its sequencer is held
        # through each transfer, and queued dma_starts would gate the first
        # EXP. Late-needed tensors issue MID-LOOP from the scalar stream
        # (see the step loop), where the issue cost hides behind exps.
        nc.sync.dma_start(kT_sb[:, 0], kT_d[0])
        nc.gpsimd.dma_start(wk[:, 0], wkT_d[:, 0])
        nc.sync.dma_start(wq[:, 0], wqT_d[:, 0])
        nc.sync.dma_start(qT_sb[:, 0], qT_d[0])
        nc.gpsimd.dma_start(wv[:, 0], wvT_d[0])
        nc.gpsimd.dma_start(vT_sb[:, 0:2], vT_d[:, 0:2])
        nc.sync.dma_start(kT_sb[:, 1], kT_d[1])
        nc.sync.dma_start(vT_sb[:, 2:5], vT_d[:, 2:5])
        nc.sync.dma_start(vT_sb[:, 5:8], vT_d[:, 5:8])
        nc.gpsimd.dma_start(
            out=bq_sb, in_=bq_d.rearrange("(m p) -> p m", p=P))
        nc.gpsimd.dma_start(wk[:, 1:MT], wkT_d[:, 1:MT])
        nc.sync.dma_start(wo, woT_d)
        for j in range(SKT):
            nc.vector.memset(vh[:, j, :, HD].bitcast(mybir.dt.uint16), 0x3C00)
        warm = rpool.tile([1, 1], F32, name="warm")
        nc.vector.memset(warm, 0.0)
        nc.scalar.activation(warm, warm, AF.Exp)
        # dummy matmuls on a zeroed tile during the DMA ramp: HAM sees a busy
        # PE and unthrottles to 2.4 GHz before the real matmuls start
        wz = persist.tile([P, NC2], F16)
        nc.vector.memset(wz, 0.0)

        def pe_warm(n):
            psw = pp.tile([P, NC2], F32, name="ppt")
            for _ in range(n):
                mm(psw, wz[:, 0:P], wz, start=True, stop=True)

        pe_warm(24)
        ones_sb = persist.tile([1, P], F16)
        nc.vector.memset(ones_sb, 1.0)

        # ---- projection groups, split into ~3-matmul units for pacing ----
        UNIT_SPLITS = ((0, 3), (3, 6), (6, 8))

        def a_half(j, h):  # v-proj: vh[:, j, heads h*4..h*4+4], one unit
            def emit():
                psa = pp.tile([P, NC2 // 2], F32, name="ppt")
                for kk in range(NT):
                    mm(psa, vT_sb[:, j, kk, :], wv[:, h, kk, :],
                       start=kk == 0, stop=kk == NT - 1)
                nc.vector.tensor_copy(
                    vh[:, j, h * 4:(h + 1) * 4, 0:HD],
                    psa.rearrange("p (h d) -> p h d", d=HD),
                )

            return [emit]

        def b_units(m, c):  # k-proj: khT[:, m, c*512:...]
            cell = {}

            def mms(lo, hi):
                if lo == 0:
                    cell["ps"] = pp.tile([P, NC2], F32, name="ppt")
                for kk in range(lo, hi):
                    mm(cell["ps"], wk[:, m, kk, :],
                       kT_sb[:, c, kk, :],
                       start=kk == 0, stop=kk == NT - 1)
                if hi == NT:
                    nc.vector.tensor_copy(
                        khT[:, m, c * NC2:(c + 1) * NC2], cell["ps"])

            return [lambda lo=lo, hi=hi: mms(lo, hi) for lo, hi in UNIT_SPLITS]

        def c_units(m, c):  # q-proj: qhT[:, m, c*512:...]
            cell = {}

            def mms(lo, hi):
                if lo == 0:
                    cell["ps"] = pp.tile([P, NC2], F32, name="ppt")
                for kk in range(lo, hi):
                    mm(cell["ps"], wq[:, m, kk, :],
                       qT_sb[:, c, kk, :],
                       start=kk == 0, stop=kk == NT - 1)
                if hi == NT:
                    nc.vector.tensor_scalar_add(
                        qhT[:, m, c * NC2:(c + 1) * NC2], cell["ps"],
                        bq_sb[:, m:m + 1])

            return [lambda lo=lo, hi=hi: mms(lo, hi) for lo, hi in UNIT_SPLITS]

        ENGS = [nc.sync, nc.scalar, nc.gpsimd]

        def e_block(sqt):  # out-proj partial rows sqt*128, both col halves
            o_sb = opool.tile([P, 2, NC2], F16, name="o_sb")
            for c in range(2):
                pse = pp.tile([P, NC2], F32, name="ppt")
                for kk in range(MT):
                    mm(pse, ctxT[:, kk, sqt * P:(sqt + 1) * P],
                       wo[:, kk, c * NC2:(c + 1) * NC2],
                       start=kk == 0, stop=kk == MT - 1)
                nc.vector.tensor_copy(o_sb[:, c, :], pse)
            # one contiguous [128, 1024] write; engines round-robin
            ENGS[sqt % 3].dma_start(out_d[sqt * P:(sqt + 1) * P, :], o_sb)

        def e_half(sqt, c):  # first half of a block as a filler unit
            def emit():
                pse = pp.tile([P, NC2], F32, name="ppt")
                for kk in range(MT):
                    mm(pse, ctxT[:, kk, sqt * P:(sqt + 1) * P],
                       wo[:, kk, c * NC2:(c + 1) * NC2],
                       start=kk == 0, stop=kk == MT - 1)
                o_sb = _osb.setdefault(
                    sqt, opool.tile([P, 2, NC2], F16, name="o_sb"))
                nc.vector.tensor_copy(o_sb[:, c, :], pse)
                if c == 1:
                    ENGS[sqt % 3].dma_start(
                        out_d[sqt * P:(sqt + 1) * P, :], o_sb)

            return [emit]

        _osb = {}

        # ---- filler stream with need-driven drains ----
        filler = []          # ordered list of (label, emit_fn)
        emitted = set()

        def add_group(label, units):
            for i, fn in enumerate(units):
                filler.append((f"{label}/{i}", fn))

        def drain_until(labels):
            todo = [x for x in labels if x not in emitted]
            if not todo:
                return
            for lbl, fn in filler:
                if lbl not in emitted:
                    emitted.add(lbl)
                    fn()
                if all(x in emitted for x in todo):
                    return

        e_ok = [False]

        def drain_next(n=1):
            done = 0
            for lbl, fn in filler:
                if lbl not in emitted:
                    if lbl.startswith("e") and not e_ok[0]:
                        return  # e-units gate on the last c=0 normalize
                    emitted.add(lbl)
                    fn()
                    done += 1
                    if done >= n:
                        return

        # ---- attention ----
        def scores(t, j, c):
            sp = pS.tile([P, 2, NC2], F32, name="sp")
            mm(sp[:, 0, :], khT[0:HD, t, j * P:(j + 1) * P],
               qhT[0:HD, t, c * NC2:(c + 1) * NC2], start=True, stop=True)
            mm(sp[:, 1, :], khT[HD:P, t, j * P:(j + 1) * P],
               qhT[HD:P, t, c * NC2:(c + 1) * NC2], start=True, stop=True)
            return sp

        def normalize(t, c, r0, r1):
            rb0 = rpool.tile([P, NC2], F32, name="rb0")
            rb1 = rpool.tile([P, NC2], F32, name="rb1")
            nc.gpsimd.partition_broadcast(rb0, r0)
            nc.gpsimd.partition_broadcast(rb1, r1)
            cs = slice(c * NC2, (c + 1) * NC2)
            nc.vector.tensor_mul(ctxT[0:HD, t, cs], ctxU[0:HD, t, cs],
                                 rb0[0:HD, :])
            nc.vector.tensor_mul(ctxT[HD:P, t, cs], ctxU[HD:P, t, cs],
                                 rb1[HD:P, :])

        # ---- emission schedule ----
        for fn in b_units(0, 0):
            fn()
        for fn in c_units(0, 0):
            fn()

        add_group("a0h0", a_half(0, 0))
        add_group("a1h0", a_half(1, 0))
        add_group("b0b", b_units(0, 1))
        add_group("c0b", c_units(0, 1))
        for j in range(2, SKT):
            add_group(f"a{j}h0", a_half(j, 0))
        add_group("b1a", b_units(1, 0))
        add_group("b1b", b_units(1, 1))
        add_group("c1a", c_units(1, 0))
        add_group("c1b", c_units(1, 1))
        add_group("b2a", b_units(2, 0))
        add_group("b2b", b_units(2, 1))
        add_group("c2a", c_units(2, 0))
        add_group("c2b", c_units(2, 1))
        for j in range(SKT):
            add_group(f"a{j}h1", a_half(j, 1))
        add_group("b3a", b_units(3, 0))
        add_group("b3b", b_units(3, 1))
        add_group("c3a", c_units(3, 0))
        add_group("c3b", c_units(3, 1))
        # first 8 output-projection groups (q-rows 0:512) become late
        # fillers: safe to drain only after normalize(3, 0) at step 58
        for sqt in range(SQ // P // 2):
            for c in range(2):
                add_group(f"e{sqt}{c}", e_half(sqt, c))

        # flat (t, c, j) pipeline, scores emitted 2 steps ahead so neither
        # PE nor ACT bubbles at block boundaries
        steps = [(t, c, j) for t in range(NPAIR) for c in range(NCQ)
                 for j in range(SKT)]
        sps = {}

        def emit_scores(idx):
            if idx >= len(steps):
                return
            t, c, j = steps[idx]
            if j == 4 and t == 0 and c == 0:
                drain_until(["b0b/2"])
            if j == 0:
                need = []
                if t >= 1:
                    need += [f"b{t}a/2", f"b{t}b/2", f"c{t}a/2", f"c{t}b/2"]
                elif c == 1:
                    need += ["c0b/2"]
                drain_until(need)
            sps[idx] = scores(t, j, c)

        pcx = {}
        rs = {}
        emit_scores(0)
        emit_scores(1)
        for idx, (t, c, j) in enumerate(steps):
            ep = epool.tile([P, 2, NC2], F16, name="ep")
            nc.scalar.activation(ep, sps.pop(idx), AF.Exp)
            # deferred input DMAs on the scalar stream: issue cost hides
            # behind the exps already queued ahead of them
            if idx == 1:
                nc.scalar.dma_start(qT_sb[:, 1], qT_d[1])
            elif idx == 3:
                nc.scalar.dma_start(wq[:, 1:MT], wqT_d[:, 1:MT])
            elif idx == 5:
                nc.scalar.dma_start(wv[:, 1], wvT_d[1])
            emit_scores(idx + 2)
            drain_until([f"a{j}h{t // 2}/0"])
            drain_next(2 if 12 <= idx < 40 else 1)
            if j == 0:
                pcx[(t, c)] = (
                    pX.tile([HD + 1, NC2], F32, name="pcx0"),
                    pX.tile([HD + 1, NC2], F32, name="pcx1"),
                )
            pcx0, pcx1 = pcx[(t, c)]
            mm(pcx0, vh[:, j, 2 * t, :], ep[:, 0, :],
               start=j == 0, stop=j == SKT - 1)
            mm(pcx1, vh[:, j, 2 * t + 1, :], ep[:, 1, :],
               start=j == 0, stop=j == SKT - 1)
            if j == SKT - 1:
                # fast PSUM drain: sum-row copies + approx reciprocals gate
                # the (deferred) normalize, so they go first on DVE
                se0 = rpool.tile([1, NC2], F32, name="se0")
                se1 = rpool.tile([1, NC2], F32, name="se1")
                nc.vector.tensor_copy(se0, pcx0[HD:HD + 1, :])
                nc.vector.tensor_copy(se1, pcx1[HD:HD + 1, :])
                r0 = rpool.tile([1, NC2], F32, name="r0")
                r1 = rpool.tile([1, NC2], F32, name="r1")
                nc.vector.reciprocal_approx_fast(r0, se0)
                nc.vector.reciprocal_approx_fast(r1, se1)
                if t == NPAIR - 1 and c == NCQ - 1:
                    r0h = rpool.tile([1, NC2], F16, name="r0h")
                    r1h = rpool.tile([1, NC2], F16, name="r1h")
                    nc.vector.tensor_copy(r0h, r0)
                    nc.vector.tensor_copy(r1h, r1)
                    rs[(t, c)] = (r0h, r1h)
                else:
                    rs[(t, c)] = (r0, r1)
                cs = slice(c * NC2, (c + 1) * NC2)
                nc.vector.tensor_copy(ctxU[0:HD, t, cs], pcx0[0:HD, :])
                nc.vector.tensor_copy(ctxU[HD:P, t, cs], pcx1[0:HD, :])
                del pcx[(t, c)]
            if j == 2 and idx >= SKT:
                pt, pc, _ = steps[idx - SKT]
                if (pt, pc) != (NPAIR - 1, NCQ - 1):
                    normalize(pt, pc, *rs.pop((pt, pc)))
                if (pt, pc) == (NPAIR - 1, 0):
                    e_ok[0] = True  # all c=0 ctxT normalized; e-units legal

        drain_until([lbl for lbl, _ in filler if not lbl.startswith("e")])

        # last block: broadcast the reciprocals on the PE (ones-row matmul
        # into a free scores-pool bank); its DVE multiply overlaps the first
        # 8 output-projection groups, which only need q-cols 0:512
        t7, c7 = NPAIR - 1, NCQ - 1
        r0, r1 = rs.pop((t7, c7))
        rb_ps = pS.tile([P, 2, NC2], F32, name="sp")
        mm(rb_ps[:, 0, :], ones_sb, r0, start=True, stop=True)
        mm(rb_ps[:, 1, :], ones_sb, r1, start=True, stop=True)
        cs = slice(c7 * NC2, (c7 + 1) * NC2)
        nc.vector.tensor_mul(ctxT[0:HD, t7, cs], ctxU[0:HD, t7, cs],
                             rb_ps[0:HD, 0, :])
        nc.vector.tensor_mul(ctxT[HD:P, t7, cs], ctxU[HD:P, t7, cs],
                             rb_ps[HD:P, 1, :])

        # ---- output projection: q-rows 0:512 ran as fillers; rest now.
        drain_until([lbl for lbl, _ in filler])
        for sqt in range(SQ // P // 2, SQ // P):
            e_block(sqt)

    nc.compile()
    return nc


def get_program():
    if "nc" not in _CACHE:
        _CACHE["nc"] = _build_program()
    return _CACHE["nc"]


def make_in_maps(q, k, v, Wq, bq, Wk, bk, Wv, bv, Wo, bo):
    f32 = lambda x: np.ascontiguousarray(np.asarray(x, dtype=np.float32))
    blk = lambda wT: np.ascontiguousarray(
        np.asarray(wT, np.float16).reshape(NT, P, MT, P).transpose(2, 1, 0, 3)
    )
    # partition-major [p, kk, w]: per-partition data is one contiguous run,
    # so each DMA descriptor covers a full row (fewer descriptors)
    pmaj = lambda xT: np.ascontiguousarray(
        np.asarray(xT, np.float16).reshape(NT, P, -1).transpose(1, 0, 2)
    )
    # q-chunk-major [c, p, kk, 512]: each c-half DMA contiguous per partition
    cmaj = lambda xT: np.ascontiguousarray(
        pmaj(xT).reshape(P, NT, NCQ, NC2).transpose(2, 0, 1, 3)
    )
    q, k, v = np.asarray(q, np.float32), np.asarray(k, np.float32), \
        np.asarray(v, np.float32)
    WqT = np.asarray(Wq, np.float32).T * np.float32(SCALE)
    WkT = np.asarray(Wk, np.float32).T
    WvT = np.asarray(Wv, np.float32).T
    WoT = np.asarray(Wo, np.float32).T
    bqs = f32(bq) * np.float32(SCALE)
    kTs = [cmaj(k[b].T) for b in range(B)]
    # vT key-tile-major: [p, j, kk, 128], same layout as the SBUF tile
    vTs = [np.ascontiguousarray(
        pmaj(v[b].T).reshape(P, NT, SKT, P).transpose(0, 2, 1, 3))
        for b in range(B)]
    qTs = [cmaj(q[b].T) for b in range(B)]
    G = MT * P                       # 512 features per head-group
    in_maps = []
    for core in range(N_CORES):
        b, g = divmod(core, 2)
        gs = slice(g * G, (g + 1) * G)
        wv_pm = pmaj(WvT[:, gs])                     # [P, NT, 512]
        wv_hh = np.ascontiguousarray(                # [2, P, NT, 256]
            wv_pm.reshape(P, NT, 2, NC2 // 2).transpose(2, 0, 1, 3))
        in_maps.append({
            "qT": qTs[b], "kT": kTs[b], "vT": vTs[b],
            "wqT": np.ascontiguousarray(
                blk(WqT[:, gs]).transpose(1, 0, 2, 3)),
            "wkT": np.ascontiguousarray(
                blk(WkT[:, gs]).transpose(1, 0, 2, 3)),
            "wvT": wv_hh,
            "woT": np.ascontiguousarray(
                np.asarray(WoT[gs, :], np.float16)
                .reshape(MT, P, D).transpose(1, 0, 2)),
            "bq": bqs[gs],
        })
    return in_maps


def gather_out(results, bo_eff):
    out = np.empty((B, S, D), dtype=np.float32)
    for b in range(B):
        out[b] = results[2 * b]["out"].astype(np.float32)
        out[b] += results[2 * b + 1]["out"].astype(np.float32)
        out[b] += bo_eff
    return out


def kernel(q, k, v, Wq, bq, Wk, bk, Wv, bv, Wo, bo):
    from concourse.bass_utils import run_bass_kernel_spmd

    nc = get_program()
    in_maps = make_in_maps(q, k, v, Wq, bq, Wk, bk, Wv, bv, Wo, bo)
    res = run_bass_kernel_spmd(nc, in_maps, list(range(N_CORES)))
    # bv folds exactly through the output projection: softmax rows sum to 1,
    # so ctx gains +bv per head, and out gains +Wo@bv. bk is irrelevant.
    bo_eff = (np.asarray(bo, np.float32)
              + np.asarray(Wo, np.float32) @ np.asarray(bv, np.float32))
    return gather_out(res.results, bo_eff)


# revision 57
# speedup vs baseline: 1.0548x; 1.0548x over previous
"""Trainium2 Bass kernel for nn_Attention (B=4, S=1024, D=1024, H=16).

Sharding: 8 cores = 4 batches x 2 head-groups (tensor parallel over heads).
Core (b, g) computes the Q/K/V projections for its 512 features (8 heads),
full-sequence attention for those heads, and a PARTIAL output projection
(contraction over its 512 ctx features). Partials are written fp16 and the
two halves of each batch are summed on the host during gather (+ bias).
No device collectives.

Device dataflow (per core) - fp16 matmul operands, fp32 PSUM accumulation:
  - host passes pre-transposed qT/kT/vT [D,S] (partition-major), m-blocked
    W{q,k}.T halves [MT,P,NT,P], Wv.T half [P,NT,512], Wo.T half [P,MT,D]
  - qhT[o,sq] = (Wq.T*SCALE).T-tiles @ qT   (o on partitions, 4 m-tiles)
  - khT[o,sk] likewise; vh[sk, h, dh] via vT-as-stationary
  - scoresT[sk,sq] per head = khT-tile.T @ qhT; the two heads of a pair run
    as K=64 matmuls at PE row strips 0:64 / 64:128 (concurrent on HW via
    per-row-tile output taps: the pair costs ~321ns, second slice ~3ns)
  - expT = exp(scoresT) on ACT (no max subtraction: |scores| < ~4)
  - ctxT_aug[dh+1, sq] += [vh | 1].T @ expT  (ones column = denominator)
  - ctx drained fast on DVE (sum-row + approx-reciprocal first), gpsimd
    broadcast + normalize multiply later, off the critical path
  - partial out[sq,o] = ctxT-tiles.T @ Wo.T-half, drained to fp16, DMA'd

The attention phase is a flat software-pipelined (pair, q-half, key-tile)
loop with scores emitted two steps ahead. Projection-matmul filler work is
split into ~3-matmul units drained one per step (need-driven ordering), so
the PE runs dense through all 64 steps instead of front-loading whole
groups and going ACT-bound at the tail. All blocks except the last
normalize in-loop; the final block's reciprocals broadcast via a PE
ones-row matmul while the first 8 output-projection groups (which only
need q-cols 0:512) run, then the rest of the output projection follows.

Bias handling (exact): bq via per-partition add on the qh copy; bk dropped
(softmax is invariant to per-query score shifts); bv folded into bo on the
host (softmax rows sum to 1); bo added host-side after summing partials.
"""

import sys

import numpy as np

if "/opt/trn_rl_repo" not in sys.path:
    sys.path.insert(0, "/opt/trn_rl_repo")

B, S, D, H = 4, 1024, 1024, 16
HD = D // H                      # 64
SCALE = 1.0 / float(np.sqrt(HD))
N_CORES = 8
SQ = S                           # full query length per core
SK = S                           # full key length
P = 128
NT = D // P                      # 8 contraction tiles
MT = 4                           # 4 out-feature tiles (512 features/core)
SKT = SK // P                    # 8 key tiles
NPAIR = 4                        # 4 head pairs per core (8 heads)
NC2 = 512                        # max matmul free dim (one PSUM bank)
NCQ = SQ // NC2                  # 2 query chunks

_CACHE = {}


def _build_program():
    from contextlib import ExitStack

    import concourse.bass as bass
    import concourse.tile as tile
    from concourse import bacc, mybir

    F32 = mybir.dt.float32
    F16 = mybir.dt.float16
    AF = mybir.ActivationFunctionType

    nc = bacc.Bacc(
        "TRN2", target_bir_lowering=False, debug=False, num_devices=N_CORES
    )

    qT_d = nc.dram_tensor("qT", [NCQ, P, NT, NC2], F16,
                          kind="ExternalInput").ap()
    kT_d = nc.dram_tensor("kT", [NCQ, P, NT, NC2], F16,
                          kind="ExternalInput").ap()
    vT_d = nc.dram_tensor("vT", [P, SKT, NT, P], F16,
                          kind="ExternalInput").ap()
    wqT_d = nc.dram_tensor("wqT", [P, MT, NT, P], F16,
                           kind="ExternalInput").ap()
    wkT_d = nc.dram_tensor("wkT", [P, MT, NT, P], F16,
                           kind="ExternalInput").ap()
    wvT_d = nc.dram_tensor("wvT", [2, P, NT, NC2 // 2], F16,
                           kind="ExternalInput").ap()
    woT_d = nc.dram_tensor("woT", [P, MT, D], F16, kind="ExternalInput").ap()
    bq_d = nc.dram_tensor("bq", [MT * P], F32, kind="ExternalInput").ap()
    out_d = nc.dram_tensor("out", [SQ, D], F16, kind="ExternalOutput").ap()

    mm = lambda *a, **k: nc.tensor.matmul(*a, **k)

    with tile.TileContext(nc) as tc, ExitStack() as ctx:
        persist = ctx.enter_context(tc.tile_pool(name="persist", bufs=1))
        epool = ctx.enter_context(tc.tile_pool(name="epool", bufs=6))
        rpool = ctx.enter_context(tc.tile_pool(name="rp", bufs=2))
        opool = ctx.enter_context(tc.tile_pool(name="outp", bufs=4))
        pp = ctx.enter_context(tc.tile_pool(name="pp", space="PSUM", bufs=2))
        pS = ctx.enter_context(tc.tile_pool(name="pS", space="PSUM", bufs=2))
        pX = ctx.enter_context(tc.tile_pool(name="pX", space="PSUM", bufs=1))

        # persistent data tiles
        qT_sb = persist.tile([P, NCQ, NT, NC2], F16)  # q-chunk-major
        kT_sb = persist.tile([P, NCQ, NT, NC2], F16)
        vT_sb = persist.tile([P, SKT, NT, P], F16)   # key-tile-major
        wq = persist.tile([P, MT, NT, P], F16)       # m-major
        wk = persist.tile([P, MT, NT, P], F16)
        wv = persist.tile([P, 2, NT, NC2 // 2], F16)  # head-half-major
        wo = persist.tile([P, MT, D], F16)
        qhT = persist.tile([P, MT, SQ], F16)        # [o'%128, o'//128, sq]
        khT = persist.tile([P, MT, SK], F16)
        vh = persist.tile([P, SKT, 8, HD + 1], F16)  # [sk%128, sk//128, h, .]
        ctxT = persist.tile([P, MT, SQ], F16)
        ctxU = persist.tile([P, MT, SQ], F16)    # unnormalized ctx (drain)
        bq_sb = persist.tile([P, MT], F32)

        # input DMAs: every transfer is contiguous on BOTH DRAM and SBUF
        # sides (2-8KB runs per partition -> few descriptors), spread across
        # the four engines' DMA paths in per-engine deadline order.
        # No pre-loop DMAs on the scalar engine: its sequencer is held
        # through each transfer, and queued dma_starts would gate the first
        # EXP. Late-needed tensors issue MID-LOOP from the scalar stream
        # (see the step loop), where the issue cost hides behind exps.
        nc.sync.dma_start(kT_sb[:, 0], kT_d[0])
        nc.gpsimd.dma_start(wk[:, 0], wkT_d[:, 0])
        nc.sync.dma_start(wq[:, 0], wqT_d[:, 0])
        nc.sync.dma_start(qT_sb[:, 0], qT_d[0])
        nc.gpsimd.dma_start(wv[:, 0], wvT_d[0])
        nc.gpsimd.dma_start(vT_sb[:, 0:2], vT_d[:, 0:2])
        nc.sync.dma_start(kT_sb[:, 1], kT_d[1])
        nc.sync.dma_start(vT_sb[:, 2:5], vT_d[:, 2:5])
        nc.sync.dma_start(vT_sb[:, 5:8], vT_d[:, 5:8])
        nc.gpsimd.dma_start(
            out=bq_sb, in_=bq_d.rearrange("(m p) -> p m", p=P))
        nc.gpsimd.dma_start(wk[:, 1:MT], wkT_d[:, 1:MT])
        nc.sync.dma_start(wo, woT_d)
        for j in range(SKT):
            nc.vector.memset(vh[:, j, :, HD].bitcast(mybir.dt.uint16), 0x3C00)
        warm = rpool.tile([1, 1], F32, name="warm")
        nc.vector.memset(warm, 0.0)
        nc.scalar.activation(warm, warm, AF.Exp)
        # dummy matmuls on a zeroed tile during the DMA ramp: HAM sees a busy
        # PE and unthrottles to 2.4 GHz before the real matmuls start
        wz = persist.tile([P, NC2], F16)
        nc.vector.memset(wz, 0.0)

        def pe_warm(n):
            psw = pp.tile([P, NC2], F32, name="ppt")
            for _ in range(n):
                mm(psw, wz[:, 0:P], wz, start=True, stop=True)

        pe_warm(24)
        ones_sb = persist.tile([1, P], F16)
        nc.vector.memset(ones_sb, 1.0)

        # ---- projection groups, split into ~3-matmul units for pacing ----
        UNIT_SPLITS = ((0, 3), (3, 6), (6, 8))

        def a_half(j, h):  # v-proj: vh[:, j, heads h*4..h*4+4], one unit
            def emit():
                psa = pp.tile([P, NC2 // 2], F32, name="ppt")
                for kk in range(NT):
                    mm(psa, vT_sb[:, j, kk, :], wv[:, h, kk, :],
                       start=kk == 0, stop=kk == NT - 1)
                nc.vector.tensor_copy(
                    vh[:, j, h * 4:(h + 1) * 4, 0:HD],
                    psa.rearrange("p (h d) -> p h d", d=HD),
                )

            return [emit]

        def b_units(m, c):  # k-proj: khT[:, m, c*512:...]
            cell = {}

            def mms(lo, hi):
                if lo == 0:
                    cell["ps"] = pp.tile([P, NC2], F32, name="ppt")
                for kk in range(lo, hi):
                    mm(cell["ps"], wk[:, m, kk, :],
                       kT_sb[:, c, kk, :],
                       start=kk == 0, stop=kk == NT - 1)
                if hi == NT:
                    nc.vector.tensor_copy(
                        khT[:, m, c * NC2:(c + 1) * NC2], cell["ps"])

            return [lambda lo=lo, hi=hi: mms(lo, hi) for lo, hi in UNIT_SPLITS]

        def c_units(m, c):  # q-proj: qhT[:, m, c*512:...]
            cell = {}

            def mms(lo, hi):
                if lo == 0:
                    cell["ps"] = pp.tile([P, NC2], F32, name="ppt")
                for kk in range(lo, hi):
                    mm(cell["ps"], wq[:, m, kk, :],
                       qT_sb[:, c, kk, :],
                       start=kk == 0, stop=kk == NT - 1)
                if hi == NT:
                    nc.vector.tensor_scalar_add(
                        qhT[:, m, c * NC2:(c + 1) * NC2], cell["ps"],
                        bq_sb[:, m:m + 1])

            return [lambda lo=lo, hi=hi: mms(lo, hi) for lo, hi in UNIT_SPLITS]

        ENGS = [nc.sync, nc.scalar, nc.gpsimd]

        def e_block(sqt):  # out-proj partial rows sqt*128, both col halves
            o_sb = opool.tile([P, 2, NC2], F16, name="o_sb")
            for c in range(2):
                pse = pp.tile([P, NC2], F32, name="ppt")
                for kk in range(MT):
                    mm(pse, ctxT[:, kk, sqt * P:(sqt + 1) * P],
                       wo[:, kk, c * NC2:(c + 1) * NC2],
                       start=kk == 0, stop=kk == MT - 1)
                nc.vector.tensor_copy(o_sb[:, c, :], pse)
            # one contiguous [128, 1024] write; engines round-robin
            ENGS[sqt % 3].dma_start(out_d[sqt * P:(sqt + 1) * P, :], o_sb)

        def e_half(sqt, c):  # first half of a block as a filler unit
            def emit():
                pse = pp.tile([P, NC2], F32, name="ppt")
                for kk in range(MT):
                    mm(pse, ctxT[:, kk, sqt * P:(sqt + 1) * P],
                       wo[:, kk, c * NC2:(c + 1) * NC2],
                       start=kk == 0, stop=kk == MT - 1)
                o_sb = _osb.setdefault(
                    sqt, opool.tile([P, 2, NC2], F16, name="o_sb"))
                nc.vector.tensor_copy(o_sb[:, c, :], pse)
                if c == 1:
                    ENGS[sqt % 3].dma_start(
                        out_d[sqt * P:(sqt + 1) * P, :], o_sb)

            return [emit]

        _osb = {}

        # ---- filler stream with need-driven drains ----
        filler = []          # ordered list of (label, emit_fn)
        emitted = set()

        def add_group(label, units):
            for i, fn in enumerate(units):
                filler.append((f"{label}/{i}", fn))

        def drain_until(labels):
            todo = [x for x in labels if x not in emitted]
            if not todo:
                return
            for lbl, fn in filler:
                if lbl not in emitted:
                    emitted.add(lbl)
                    fn()
                if all(x in emitted for x in todo):
                    return

        e_ok = [False]

        def drain_next(n=1):
            done = 0
            for lbl, fn in filler:
                if lbl not in emitted:
                    if lbl.startswith("e") and not e_ok[0]:
                        return  # e-units gate on the last c=0 normalize
                    emitted.add(lbl)
                    fn()
                    done += 1
                    if done >= n:
                        return

        # ---- attention ----
        def scores(t, j, c):
            sp = pS.tile([P, 2, NC2], F32, name="sp")
            mm(sp[:, 0, :], khT[0:HD, t, j * P:(j + 1) * P],
               qhT[0:HD, t, c * NC2:(c + 1) * NC2], start=True, stop=True)
            mm(sp[:, 1, :], khT[HD:P, t, j * P:(j + 1) * P],
               qhT[HD:P, t, c * NC2:(c + 1) * NC2], start=True, stop=True)
            return sp

        def normalize(t, c, r0, r1):
            rb0 = rpool.tile([P, NC2], F32, name="rb0")
            rb1 = rpool.tile([P, NC2], F32, name="rb1")
            nc.gpsimd.partition_broadcast(rb0, r0)
            nc.gpsimd.partition_broadcast(rb1, r1)
            cs = slice(c * NC2, (c + 1) * NC2)
            nc.vector.tensor_mul(ctxT[0:HD, t, cs], ctxU[0:HD, t, cs],
                                 rb0[0:HD, :])
            nc.vector.tensor_mul(ctxT[HD:P, t, cs], ctxU[HD:P, t, cs],
                                 rb1[HD:P, :])

        # ---- emission schedule ----
        for fn in b_units(0, 0):
            fn()
        for fn in c_units(0, 0):
            fn()

        add_group("a0h0", a_half(0, 0))
        add_group("a1h0", a_half(1, 0))
        add_group("b0b", b_units(0, 1))
        add_group("c0b", c_units(0, 1))
        for j in range(2, SKT):
            add_group(f"a{j}h0", a_half(j, 0))
        add_group("b1a", b_units(1, 0))
        add_group("b1b", b_units(1, 1))
        add_group("c1a", c_units(1, 0))
        add_group("c1b", c_units(1, 1))
        add_group("b2a", b_units(2, 0))
        add_group("b2b", b_units(2, 1))
        add_group("c2a", c_units(2, 0))
        add_group("c2b", c_units(2, 1))
        for j in range(SKT):
            add_group(f"a{j}h1", a_half(j, 1))
        add_group("b3a", b_units(3, 0))
        add_group("b3b", b_units(3, 1))
        add_group("c3a", c_units(3, 0))
        add_group("c3b", c_units(3, 1))
        # first 8 output-projection groups (q-rows 0:512) become late
        # fillers: safe to drain only after normalize(3, 0) at step 58
        for sqt in range(SQ // P // 2):
            for c in range(2):
                add_group(f"e{sqt}{c}", e_half(sqt, c))

        # flat (t, c, j) pipeline, scores emitted 2 steps ahead so neither
        # PE nor ACT bubbles at block boundaries
        steps = [(t, c, j) for t in range(NPAIR) for c in range(NCQ)
                 for j in range(SKT)]
        sps = {}

        def emit_scores(idx):
            if idx >= len(steps):
                return
            t, c, j = steps[idx]
            if j == 4 and t == 0 and c == 0:
                drain_until(["b0b/2"])
            if j == 0:
                need = []
                if t >= 1:
                    need += [f"b{t}a/2", f"b{t}b/2", f"c{t}a/2", f"c{t}b/2"]
                elif c == 1:
                    need += ["c0b/2"]
                drain_until(need)
            sps[idx] = scores(t, j, c)

        pcx = {}
        rs = {}
        emit_scores(0)
        emit_scores(1)
        for idx, (t, c, j) in enumerate(steps):
            ep = epool.tile([P, 2, NC2], F16, name="ep")
            nc.scalar.activation(ep, sps.pop(idx), AF.Exp)
            # deferred input DMAs on the scalar stream: issue cost hides
            # behind the exps already queued ahead of them
            if idx == 1:
                nc.scalar.dma_start(qT_sb[:, 1], qT_d[1])
            elif idx == 3:
                nc.scalar.dma_start(wq[:, 1:MT], wqT_d[:, 1:MT])
            elif idx == 5:
                nc.scalar.dma_start(wv[:, 1], wvT_d[1])
            emit_scores(idx + 2)
            drain_until([f"a{j}h{t // 2}/0"])
            drain_next(1)
            if j == 0:
                pcx[(t, c)] = (
                    pX.tile([HD + 1, NC2], F32, name="pcx0"),
                    pX.tile([HD + 1, NC2], F32, name="pcx1"),
                )
            pcx0, pcx1 = pcx[(t, c)]
            mm(pcx0, vh[:, j, 2 * t, :], ep[:, 0, :],
               start=j == 0, stop=j == SKT - 1)
            mm(pcx1, vh[:, j, 2 * t + 1, :], ep[:, 1, :],
               start=j == 0, stop=j == SKT - 1)
            if j == SKT - 1:
                # fast PSUM drain: sum-row copies + approx reciprocals gate
                # the (deferred) normalize, so they go first on DVE
                se0 = rpool.tile([1, NC2], F32, name="se0")
                se1 = rpool.tile([1, NC2], F32, name="se1")
                nc.vector.tensor_copy(se0, pcx0[HD:HD + 1, :])
                nc.vector.tensor_copy(se1, pcx1[HD:HD + 1, :])
                r0 = rpool.tile([1, NC2], F32, name="r0")
                r1 = rpool.tile([1, NC2], F32, name="r1")
                nc.vector.reciprocal_approx_fast(r0, se0)
                nc.vector.reciprocal_approx_fast(r1, se1)
                if t == NPAIR - 1 and c == NCQ - 1:
                    r0h = rpool.tile([1, NC2], F16, name="r0h")
                    r1h = rpool.tile([1, NC2], F16, name="r1h")
                    nc.vector.tensor_copy(r0h, r0)
                    nc.vector.tensor_copy(r1h, r1)
                    rs[(t, c)] = (r0h, r1h)
                else:
                    rs[(t, c)] = (r0, r1)
                cs = slice(c * NC2, (c + 1) * NC2)
                nc.vector.tensor_copy(ctxU[0:HD, t, cs], pcx0[0:HD, :])
                nc.vector.tensor_copy(ctxU[HD:P, t, cs], pcx1[0:HD, :])
                del pcx[(t, c)]
            if j == 2 and idx >= SKT:
                pt, pc, _ = steps[idx - SKT]
                if (pt, pc) != (NPAIR - 1, NCQ - 1):
                    normalize(pt, pc, *rs.pop((pt, pc)))
                if (pt, pc) == (NPAIR - 1, 0):
                    e_ok[0] = True  # all c=0 ctxT normalized; e-units legal

        drain_until([lbl for lbl, _ in filler if not lbl.startswith("e")])

        # last block: broadcast the reciprocals on the PE (ones-row matmul
        # into a free scores-pool bank); its DVE multiply overlaps the first
        # 8 output-projection groups, which only need q-cols 0:512
        t7, c7 = NPAIR - 1, NCQ - 1
        r0, r1 = rs.pop((t7, c7))
        rb_ps = pS.tile([P, 2, NC2], F32, name="sp")
        mm(rb_ps[:, 0, :], ones_sb, r0, start=True, stop=True)
        mm(rb_ps[:, 1, :], ones_sb, r1, start=True, stop=True)
        cs = slice(c7 * NC2, (c7 + 1) * NC2)
        nc.vector.tensor_mul(ctxT[0:HD, t7, cs], ctxU[0:HD, t7, cs],
                             rb_ps[0:HD, 0, :])
        nc.vector.tensor_mul(ctxT[HD:P, t7, cs], ctxU[HD:P, t7, cs],
                             rb_ps[HD:P, 1, :])

        # ---- output projection: q-rows 0:512 ran as fillers; rest now.
        drain_until([lbl for lbl, _ in filler])
        for sqt in range(SQ // P // 2, SQ // P):
            e_block(sqt)

    nc.compile()
    return nc


def get_program():
    if "nc" not in _CACHE:
        _CACHE["nc"] = _build_program()
    return _CACHE["nc"]


def make_in_maps(q, k, v, Wq, bq, Wk, bk, Wv, bv, Wo, bo):
    f32 = lambda x: np.ascontiguousarray(np.asarray(x, dtype=np.float32))
    blk = lambda wT: np.ascontiguousarray(
        np.asarray(wT, np.float16).reshape(NT, P, MT, P).transpose(2, 1, 0, 3)
    )
    # partition-major [p, kk, w]: per-partition data is one contiguous run,
    # so each DMA descriptor covers a full row (fewer descriptors)
    pmaj = lambda xT: np.ascontiguousarray(
        np.asarray(xT, np.float16).reshape(NT, P, -1).transpose(1, 0, 2)
    )
    # q-chunk-major [c, p, kk, 512]: each c-half DMA contiguous per partition
    cmaj = lambda xT: np.ascontiguousarray(
        pmaj(xT).reshape(P, NT, NCQ, NC2).transpose(2, 0, 1, 3)
    )
    q, k, v = np.asarray(q, np.float32), np.asarray(k, np.float32), \
        np.asarray(v, np.float32)
    WqT = np.asarray(Wq, np.float32).T * np.float32(SCALE)
    WkT = np.asarray(Wk, np.float32).T
    WvT = np.asarray(Wv, np.float32).T
    WoT = np.asarray(Wo, np.float32).T
    bqs = f32(bq) * np.float32(SCALE)
    kTs = [cmaj(k[b].T) for b in range(B)]
    # vT key-tile-major: [p, j, kk, 128], same layout as the SBUF tile
    vTs = [np.ascontiguousarray(
        pmaj(v[b].T).reshape(P, NT, SKT, P).transpose(0, 2, 1, 3))
        for b in range(B)]
    qTs = [cmaj(q[b].T) for b in range(B)]
    G = MT * P                       # 512 features per head-group
    in_maps = []
    for core in range(N_CORES):
        b, g = divmod(core, 2)
        gs = slice(g * G, (g + 1) * G)
        wv_pm = pmaj(WvT[:, gs])                     # [P, NT, 512]
        wv_hh = np.ascontiguousarray(                # [2, P, NT, 256]
            wv_pm.reshape(P, NT, 2, NC2 // 2).transpose(2, 0, 1, 3))
        in_maps.append({
            "qT": qTs[b], "kT": kTs[b], "vT": vTs[b],
            "wqT": np.ascontiguousarray(
                blk(WqT[:, gs]).transpose(1, 0, 2, 3)),
            "wkT": np.ascontiguousarray(
                blk(WkT[:, gs]).transpose(1, 0, 2, 3)),
            "wvT": wv_hh,
            "woT": np.ascontiguousarray(
                np.asarray(WoT[gs, :], np.float16)
                .reshape(MT, P, D).transpose(1, 0, 2)),
            "bq": bqs[gs],
        })
    return in_maps


def gather_out(results, bo_eff):
    out = np.empty((B, S, D), dtype=np.float32)
    for b in range(B):
        out[b] = results[2 * b]["out"].astype(np.float32)
        out[b] += results[2 * b + 1]["out"].astype(np.float32)
        out[b] += bo_eff
    return out


def kernel(q, k, v, Wq, bq, Wk, bk, Wv, bv, Wo, bo):
    from concourse.bass_utils import run_bass_kernel_spmd

    nc = get_program()
    in_maps = make_in_maps(q, k, v, Wq, bq, Wk, bk, Wv, bv, Wo, bo)
    res = run_bass_kernel_spmd(nc, in_maps, list(range(N_CORES)))
    # bv folds exactly through the output projection: softmax rows sum to 1,
    # so ctx gains +bv per head, and out gains +Wo@bv. bk is irrelevant.
    bo_eff = (np.asarray(bo, np.float32)
              + np.asarray(Wo, np.float32) @ np.asarray(bv, np.float32))
    return gather_out(res.results, bo_eff)
